# revision 1
# baseline (speedup 1.0000x reference)
"""Depthwise cross-correlation (SiamFC-style) Trainium2 kernel.

z: [128, 256, 7, 7] templates, x: [128, 256, 31, 31] search images.
out[b,c,p,q] = sum_{i,j} z[b,c,i,j] * x[b,c,p+i,q+j]  -> [128, 256, 25, 25]

Strategy: pure data parallel over batch (16 batches/core on 8 cores).
Per core: 4096 (b,c) channel pairs = 32 tiles of 128 partitions.
Each tile: 49 diagonal-weight matmuls (one per filter tap) accumulate in
PSUM; rhs is a strided window view of the naturally-laid-out x tile, so
no data replication is needed. float32r streams at 1 col/cycle for N>=256.
"""

import numpy as np

B, C = 128, 256
HZ, WZ = 7, 7
HX, WX = 31, 31
HO, WO = 25, 25
N_CORES = 8
B_PER_CORE = B // N_CORES            # 16
PAIRS = B_PER_CORE * C               # 4096 channel pairs per core
NTILES = PAIRS // 128                # 32
XF = HX * WX                         # 961
ZF = HZ * WZ                         # 49
OF = HO * WO                         # 625
# output p-chunks so each PSUM chunk is one bank (<=512 f32) and N>=256.
# fp32r ISA: innermost free count must be even (moving + psum dst), outer
# steps even -> pad the q-window to 26 (extra col discarded) and pad x rows
# to 32 cols so window reads stay in-tile.
WQ = 26                              # padded q-window (col 25 is garbage)
WXP = 32                             # padded x row pitch
P_SPLIT = 13                         # chunk A: p in [0,13) -> N=338; B: 12*26=312
NA = P_SPLIT * WQ
NB = (HO - P_SPLIT) * WQ
# engine balance: taps 0..ZF-K_OFF-1 run on PE (diag matmul); the last
# K_OFF taps run as ACT products + DVE accumulate. Diag weight builds are
# split DVE/ACT.
K_OFF = 10
N_BUILD_ACT = 12
# emit all chunk-A matmuls, then all chunk-B (avoids PSUM bank ping-pong
# between consecutive MMs); weights stay resident across both passes.
SPLIT_AB = False


def _install_tilefix():
    """This walrus build accepts only one sync-wait command on a Drain.
    Split the TileContext tail-drain waits across single-wait SP nops."""
    import concourse.tile as tile_mod
    from concourse.vector_clock import ScopedClock

    def _drain_and_barrier_split(self, tick_clock, wait_clock):
        nc = self.nc
        probe = nc.sync.nop(nofuse=True, hint="drain_wait_probe")
        wait_clock.add_sem_waits(
            probe.ins, ScopedClock({None: tick_clock.global_clock})
        )
        si = probe.ins.sync_info
        waits = list(si.on_wait) if si is not None and si.on_wait else []
        if si is not None:
            si.on_wait = waits[:1]
        for w in waits[1:]:
            stub = nc.sync.nop(nofuse=True, hint="drain_wait_split")
            ssi = stub.ins.sync_info
            if ssi is None:
                import concourse.mybir as mybir
                stub.ins.sync_info = mybir.SyncInfo(on_wait=[w], on_update=[])
            else:
                ssi.on_wait = list(ssi.on_wait or []) + [w]
        nc.sync.drain()
        nc.all_engine_barrier()
        assert self.sems is not None
        popped = nc._tile_sem_poison_stack.pop()
        assert popped is self._sem_poison
        nc.clear_and_free_semaphores(list(self.sems.allocated().values()))
        nc.all_engine_barrier()

    tile_mod.TileContext._drain_and_barrier = _drain_and_barrier_split


def _split_multi_waits(nc):
    """This walrus build accepts only one sync-wait command per instruction.
    Hoist extra waits onto single-wait nops on the same engine just before."""
    import concourse.mybir as mybir

    n = 0
    for f in nc.m.functions:
        for bb in f.blocks:
            insts = list(bb.instructions)
            out_insts = []
            changed = False
            for inst in insts:
                si = inst.sync_info
                if si is not None and si.on_wait and len(si.on_wait) > 1:
                    waits = list(si.on_wait)
                    si.on_wait = waits[-1:]
                    for w in waits[:-1]:
                        n += 1
                        out_insts.append(mybir.InstNoOp(
                            name=f"waitsplit-{n}",
                            engine=inst.engine,
                            bass_nofuse=True,
                            sync_info=mybir.SyncInfo(on_wait=[w], on_update=[]),
                        ))
                    changed = True
                out_insts.append(inst)
            if changed:
                bb.instructions.clear()
                for inst in out_insts:
                    bb.add_instruction(inst)
    return n


_NC_CACHE = {}


def _build_bass(reps: int = 1, timing: bool = False):
    import concourse.bass as bass
    import concourse.mybir as mybir
    import concourse.tile as tile
    from concourse.masks import make_identity
    from contextlib import ExitStack

    _install_tilefix()

    f32 = mybir.dt.float32
    f32r = mybir.dt.float32r

    nc = bass.Bass()
    xs = nc.declare_dram_parameter("xs", [PAIRS, XF], f32, isOutput=False)
    zs = nc.declare_dram_parameter("zs", [PAIRS, ZF], f32, isOutput=False)
    out_rows = 128 if timing else PAIRS
    out = nc.declare_dram_parameter("out", [out_rows, OF], f32, isOutput=True)

    with tile.TileContext(nc) as tc:
        with (
            tc.tile_pool(name="consts", bufs=1) as consts,
            tc.tile_pool(name="xin", bufs=3) as xin,
            tc.tile_pool(name="zin", bufs=3) as zin,
            tc.tile_pool(name="wts", bufs=(44 if SPLIT_AB else 12)) as wts,
            tc.tile_pool(name="outp", bufs=3) as outp,
            tc.tile_pool(name="accp", bufs=2) as accp,
            tc.tile_pool(name="prodp", bufs=3) as prodp,
            tc.tile_pool(name="psum", bufs=3, space="PSUM") as psum,
        ):
            ident = consts.tile([128, 128], f32)
            make_identity(nc, ident)

            # x is stored row-pitch-32 (even outer stride — the fp32r moving
            # path faults on odd outer strides even though the walrus
            # verifier only checks the innermost dim).
            def win(x_t, i, j, p0, pc, wq):
                return x_t[:, i + p0:i + p0 + pc, j:j + wq]

            for _rep in range(reps):
              for t in range(NTILES):
                r0 = t * 128
                x_t = xin.tile([128, HX, WXP], f32r)
                nc.gpsimd.dma_start(
                    out=x_t[:, :, 0:WX],
                    in_=xs[r0:r0 + 128, :].rearrange("p (h w) -> p h w", h=HX))
                z_t = zin.tile([128, ZF], f32)
                nc.sync.dma_start(out=z_t, in_=zs[r0:r0 + 128, :])

                ps_a = psum.tile([128, P_SPLIT, WQ], f32)
                ps_b = psum.tile([128, HO - P_SPLIT, WQ], f32)

                n_pe = ZF - K_OFF
                w_tiles = []
                for tap in range(n_pe):
                    i, j = divmod(tap, WZ)
                    w = wts.tile([128, 128], f32r)
                    zcol = z_t[:, tap:tap + 1]
                    if tap % 3 == 1 and tap // 3 < N_BUILD_ACT:
                        nc.scalar.mul(w, ident, zcol)
                    else:
                        nc.vector.tensor_scalar_mul(w, ident, zcol)
                    w_tiles.append(w)
                    rhs_a = win(x_t, i, j, 0, P_SPLIT, WQ)
                    nc.tensor.matmul(
                        ps_a, w, rhs_a,
                        start=(tap == 0), stop=(tap == n_pe - 1),
                        skip_group_check=True,
                    )
                    if not SPLIT_AB:
                        rhs_b = win(x_t, i, j, P_SPLIT, HO - P_SPLIT, WQ)
                        nc.tensor.matmul(
                            ps_b, w, rhs_b,
                            start=(tap == 0), stop=(tap == n_pe - 1),
                            skip_group_check=True,
                        )
                if SPLIT_AB:
                    for tap in range(n_pe):
                        i, j = divmod(tap, WZ)
                        rhs_b = win(x_t, i, j, P_SPLIT, HO - P_SPLIT, WQ)
                        nc.tensor.matmul(
                            ps_b, w_tiles[tap], rhs_b,
                            start=(tap == 0), stop=(tap == n_pe - 1),
                            skip_group_check=True,
                        )

                # offloaded taps: ACT per-partition-scaled product, DVE adds
                acc = accp.tile([128, HO, WO], f32)
                x_f = x_t.bitcast(f32)
                for n, tap in enumerate(range(n_pe, ZF)):
                    i, j = divmod(tap, WZ)
                    x_win = win(x_f, i, j, 0, HO, WO)
                    zcol = z_t[:, tap:tap + 1]
                    if n == 0:
                        nc.scalar.mul(acc, x_win, zcol)
                    else:
                        prod = prodp.tile([128, HO, WO], f32)
                        nc.scalar.mul(prod, x_win, zcol)
                        nc.vector.tensor_add(acc, acc, prod)

                o_t = outp.tile([128, HO, WO], f32)
                nc.vector.tensor_add(
                    o_t[:, 0:P_SPLIT, :], acc[:, 0:P_SPLIT, :],
                    ps_a[:, :, 0:WO])
                nc.vector.tensor_add(
                    o_t[:, P_SPLIT:HO, :], acc[:, P_SPLIT:HO, :],
                    ps_b[:, :, 0:WO])
                o0 = 0 if timing else r0
                nc.sync.dma_start(
                    out=out[o0:o0 + 128, :],
                    in_=o_t.rearrange("p h w -> p (h w)"))

    _split_multi_waits(nc)
    return nc


def _get_nc(reps: int = 1, timing: bool = False):
    key = ("nc", reps, timing)
    if key not in _NC_CACHE:
        _NC_CACHE[key] = _build_bass(reps, timing)
    return _NC_CACHE[key]


def kernel(z: np.ndarray, x: np.ndarray, _trace: bool = False):
    from concourse.bass_utils import run_bass_kernel_spmd

    z = np.ascontiguousarray(z, dtype=np.float32)
    x = np.ascontiguousarray(x, dtype=np.float32)
    assert z.shape == (B, C, HZ, WZ) and x.shape == (B, C, HX, WX)

    nc = _get_nc()
    in_maps = []
    for c in range(N_CORES):
        b0 = c * B_PER_CORE
        in_maps.append({
            "xs": x[b0:b0 + B_PER_CORE].reshape(PAIRS, XF),
            "zs": z[b0:b0 + B_PER_CORE].reshape(PAIRS, ZF),
        })
    res = run_bass_kernel_spmd(nc, in_maps, list(range(N_CORES)), trace=_trace)
    out = np.empty((B, C, HO, WO), dtype=np.float32)
    for c in range(N_CORES):
        b0 = c * B_PER_CORE
        out[b0:b0 + B_PER_CORE] = res.results[c]["out"].reshape(
            B_PER_CORE, C, HO, WO)
    if _trace:
        return out, res
    return out



# revision 4
# speedup vs baseline: 2658.4736x; 2658.4736x over previous
"""Depthwise cross-correlation (SiamFC-style) Trainium2 kernel, v2.

z: [128, 256, 7, 7] templates, x: [128, 256, 31, 31] search images.
out[b,c,p,q] = sum_{i,j} z[b,c,i,j] * x[b,c,p+i,q+j]  -> [128, 256, 25, 25]

Data parallel over batch (16 batches/core on 8 cores); per core 4096
(b,c) pairs = 32 tiles of 128 partitions.

v2 strategy (vs v1 single-stream fp32r diag trick):
- everything bf16 on-chip (SWDGE DMA casts fp32->bf16 on load; tolerance
  is 2e-2, bf16 error ~3e-3).
- PE column tiling with TWO concurrent moving streams: stream A covers
  pairs 0-63 (psum partitions 0-63), stream B pairs 64-127.  Each
  stream's rhs tile holds its 64 pairs twice: partitions [0:64] natural,
  [64:128] shifted down one image row.  A [128,64] stacked-diagonal
  weight (diag(z[tap]) over rows 0:64 + diag(z[tap+row]) over 64:128)
  then makes ONE matmul accumulate TWO filter taps for 64 pairs, and the
  two streams run concurrently in separate column groups -> 4 taps per
  625-cycle column sweep = ~2x the v1 PE rate.
- 21 tap-pair groups (rows 0-5 x 7 cols) on PE; the 7 row-6 taps are
  offloaded: even-col ones to DVE fused scalar_tensor_tensor MACs,
  odd-col ones (2-byte-misaligned windows, DVE would drop to 1x) to ACT
  mul + DVE add.
- weight builds: stacked-identity * z-column (tensor_scalar_mul, 4x bf16
  mode on DVE / dtype-independent on ACT), split between ACT and DVE to
  balance the engines.
"""

import numpy as np

B, C = 128, 256
HZ, WZ = 7, 7
HX, WX = 31, 31
HO, WO = 25, 25
N_CORES = 8
B_PER_CORE = B // N_CORES            # 16
PAIRS = B_PER_CORE * C               # 4096 channel pairs per core
NTILES = PAIRS // 128                # 32
XF = HX * WX                         # 961
ZF = HZ * WZ                         # 49
OF = HO * WO                         # 625
import os
WQ = int(os.environ.get("K_WQ", "26"))   # q-window; 26 pads col 25 (garbage)
WXP = 32                             # padded x row pitch
P_SPLIT = 13                         # psum chunk A rows; B gets 12
# PE tap-pair groups: (i0, jc) covers taps (i0, jc) and (i0+1, jc)
PE_GROUPS = [(i0, jc) for i0 in (0, 2, 4) for jc in range(7)]
# offloaded taps: row 6.  even jc -> DVE fused MAC; odd jc -> ACT mul.
OFF_DVE = [(6, jc) for jc in (0, 2, 4, 6)]
OFF_ACT = [(6, jc) for jc in (1, 3, 5)]
N_BUILD_ACT = 11                     # of the 42 weight builds, this many on ACT


def _install_tilefix():
    """This walrus build accepts only one sync-wait command on a Drain.
    Split the TileContext tail-drain waits across single-wait SP nops."""
    import concourse.tile as tile_mod
    from concourse.vector_clock import ScopedClock

    def _drain_and_barrier_split(self, tick_clock, wait_clock):
        nc = self.nc
        probe = nc.sync.nop(nofuse=True, hint="drain_wait_probe")
        wait_clock.add_sem_waits(
            probe.ins, ScopedClock({None: tick_clock.global_clock})
        )
        si = probe.ins.sync_info
        waits = list(si.on_wait) if si is not None and si.on_wait else []
        if si is not None:
            si.on_wait = waits[:1]
        for w in waits[1:]:
            stub = nc.sync.nop(nofuse=True, hint="drain_wait_split")
            ssi = stub.ins.sync_info
            if ssi is None:
                import concourse.mybir as mybir
                stub.ins.sync_info = mybir.SyncInfo(on_wait=[w], on_update=[])
            else:
                ssi.on_wait = list(ssi.on_wait or []) + [w]
        nc.sync.drain()
        nc.all_engine_barrier()
        assert self.sems is not None
        popped = nc._tile_sem_poison_stack.pop()
        assert popped is self._sem_poison
        nc.clear_and_free_semaphores(list(self.sems.allocated().values()))
        nc.all_engine_barrier()

    tile_mod.TileContext._drain_and_barrier = _drain_and_barrier_split


def _split_multi_waits(nc):
    """This walrus build accepts only one sync-wait command per instruction.
    Hoist extra waits onto single-wait nops on the same engine just before."""
    import concourse.mybir as mybir

    n = 0
    for f in nc.m.functions:
        for bb in f.blocks:
            insts = list(bb.instructions)
            out_insts = []
            changed = False
            for inst in insts:
                si = inst.sync_info
                if si is not None and si.on_wait and len(si.on_wait) > 1:
                    waits = list(si.on_wait)
                    si.on_wait = waits[-1:]
                    for w in waits[:-1]:
                        n += 1
                        out_insts.append(mybir.InstNoOp(
                            name=f"waitsplit-{n}",
                            engine=inst.engine,
                            bass_nofuse=True,
                            sync_info=mybir.SyncInfo(on_wait=[w], on_update=[]),
                        ))
                    changed = True
                out_insts.append(inst)
            if changed:
                bb.instructions.clear()
                for inst in out_insts:
                    bb.add_instruction(inst)
    return n


_NC_CACHE = {}


def _build_bass(reps: int = 1, timing: bool = False):
    import concourse.bass as bass
    import concourse.mybir as mybir
    import concourse.tile as tile
    from contextlib import ExitStack

    _install_tilefix()

    f32 = mybir.dt.float32
    bf16 = mybir.dt.bfloat16
    mult = mybir.AluOpType.mult
    add = mybir.AluOpType.add

    nc = bass.Bass()
    xs = nc.declare_dram_parameter("xs", [PAIRS, XF], f32, isOutput=False)
    zs = nc.declare_dram_parameter("zs", [PAIRS, ZF], f32, isOutput=False)
    out_rows = 128 if timing else PAIRS
    out = nc.declare_dram_parameter("out", [out_rows, OF], f32, isOutput=True)

    with tile.TileContext(nc) as tc:
        with (
            tc.tile_pool(name="consts", bufs=1) as consts,
            tc.tile_pool(name="xin", bufs=3) as xin,
            tc.tile_pool(name="xrep", bufs=2) as xrep,
            tc.tile_pool(name="zin", bufs=3) as zin,
            tc.tile_pool(name="zrep", bufs=2) as zrep,
            tc.tile_pool(name="wts", bufs=24) as wts,
            tc.tile_pool(name="outp", bufs=3) as outp,
            tc.tile_pool(name="accp", bufs=2) as accp,
            tc.tile_pool(name="prodp", bufs=3) as prodp,
            tc.tile_pool(name="psum", bufs=3, space="PSUM") as psum,
        ):
            # stacked identity: I2[p, c] = 1 where p == c or p == c + 64
            i2 = consts.tile([128, 64], bf16)
            nc.gpsimd.memset(i2, 0.0)
            nc.gpsimd.affine_select(
                out=i2, in_=i2, compare_op=mybir.AluOpType.not_equal,
                fill=1.0, base=0, pattern=[[-1, 64]], channel_multiplier=1)
            nc.gpsimd.affine_select(
                out=i2, in_=i2, compare_op=mybir.AluOpType.not_equal,
                fill=1.0, base=-64, pattern=[[-1, 64]], channel_multiplier=1)

            for _rep in range(reps):
              for t in range(NTILES):
                r0 = t * 128
                # natural-layout x (for the offloaded row-6 taps), bf16
                x_t = xin.tile([128, HX, WXP], bf16)
                nc.gpsimd.dma_start(
                    out=x_t[:, :, 0:WX],
                    in_=xs[r0:r0 + 128, :].rearrange("p (h w) -> p h w", h=HX))
                z_t = zin.tile([128, ZF], f32)
                nc.sync.dma_start(out=z_t, in_=zs[r0:r0 + 128, :])

                # stream tiles: xa = pairs 0-63 (rows natural | shifted 1
                # image row), xb = pairs 64-127 likewise.
                xa = xrep.tile([128, HX, WXP], bf16)
                xb = xrep.tile([128, HX, WXP], bf16)
                nc.sync.dma_start(out=xa[0:64], in_=x_t[0:64])
                nc.sync.dma_start(out=xa[64:128, 0:HX - 1, :],
                                  in_=x_t[0:64, 1:HX, :])
                nc.sync.dma_start(out=xb[0:64], in_=x_t[64:128])
                nc.sync.dma_start(out=xb[64:128, 0:HX - 1, :],
                                  in_=x_t[64:128, 1:HX, :])
                # z replicas: [64:128] tap-shifted by one filter row (+WZ)
                # so one column of za/zb holds both taps of a group.
                za = zrep.tile([128, ZF], f32)
                zb = zrep.tile([128, ZF], f32)
                nc.sync.dma_start(out=za[0:64], in_=z_t[0:64])
                nc.sync.dma_start(out=za[64:128, 0:ZF - WZ],
                                  in_=z_t[0:64, WZ:ZF])
                nc.sync.dma_start(out=zb[0:64], in_=z_t[64:128])
                nc.sync.dma_start(out=zb[64:128, 0:ZF - WZ],
                                  in_=z_t[64:128, WZ:ZF])

                ps_a = psum.tile([128, P_SPLIT, WQ], f32)
                ps_b = psum.tile([128, HO - P_SPLIT, WQ], f32)

                ng = len(PE_GROUPS)
                for g, (i0, jc) in enumerate(PE_GROUPS):
                    goff = WZ * i0 + jc
                    wa = wts.tile([128, 64], bf16)
                    wb = wts.tile([128, 64], bf16)
                    if (2 * g) % ng < N_BUILD_ACT:
                        nc.scalar.mul(wa, i2, za[:, goff:goff + 1])
                    else:
                        nc.vector.tensor_scalar_mul(wa, i2, za[:, goff:goff + 1])
                    if (2 * g + 1) % ng < N_BUILD_ACT:
                        nc.scalar.mul(wb, i2, zb[:, goff:goff + 1])
                    else:
                        nc.vector.tensor_scalar_mul(wb, i2, zb[:, goff:goff + 1])
                    first = g == 0
                    last = g == ng - 1
                    rhs_aa = xa[:, i0:i0 + P_SPLIT, jc:jc + WQ]
                    rhs_ab = xa[:, i0 + P_SPLIT:i0 + HO, jc:jc + WQ]
                    rhs_ba = xb[:, i0:i0 + P_SPLIT, jc:jc + WQ]
                    rhs_bb = xb[:, i0 + P_SPLIT:i0 + HO, jc:jc + WQ]
                    nc.tensor.matmul(ps_a[0:64], wa, rhs_aa,
                                     start=first, stop=last,
                                     skip_group_check=True)
                    nc.tensor.matmul(ps_b[0:64], wa, rhs_ab,
                                     start=first, stop=last,
                                     skip_group_check=True)
                    nc.tensor.matmul(ps_a[64:128], wb, rhs_ba,
                                     start=first, stop=last,
                                     skip_group_check=True)
                    nc.tensor.matmul(ps_b[64:128], wb, rhs_bb,
                                     start=first, stop=last,
                                     skip_group_check=True)

                # offloaded row-6 taps, elementwise over all 128 pairs
                acc = accp.tile([128, HO, WO], bf16)
                prods = []
                for (i, jc) in OFF_ACT:
                    zcol = z_t[:, WZ * i + jc:WZ * i + jc + 1]
                    x_win = x_t[:, i:i + HO, jc:jc + WO]
                    prod = prodp.tile([128, HO, WO], bf16)
                    nc.scalar.mul(prod, x_win, zcol)
                    prods.append(prod)
                for n, (i, jc) in enumerate(OFF_DVE):
                    zcol = z_t[:, WZ * i + jc:WZ * i + jc + 1]
                    x_win = x_t[:, i:i + HO, jc:jc + WO]
                    if n == 0:
                        nc.vector.tensor_scalar_mul(acc, x_win, zcol)
                    else:
                        nc.vector.scalar_tensor_tensor(
                            acc, x_win, zcol, acc, op0=mult, op1=add)
                for prod in prods:
                    nc.vector.tensor_add(acc, acc, prod)

                # merge psum + acc -> fp32 output
                o_t = outp.tile([128, HO, WO], f32)
                nc.vector.scalar_tensor_tensor(
                    o_t[:, 0:P_SPLIT, :], acc[:, 0:P_SPLIT, :], 1.0,
                    ps_a[:, :, 0:WO], op0=mult, op1=add)
                nc.vector.scalar_tensor_tensor(
                    o_t[:, P_SPLIT:HO, :], acc[:, P_SPLIT:HO, :], 1.0,
                    ps_b[:, :, 0:WO], op0=mult, op1=add)
                o0 = 0 if timing else r0
                nc.sync.dma_start(
                    out=out[o0:o0 + 128, :],
                    in_=o_t.rearrange("p h w -> p (h w)"))

    _split_multi_waits(nc)
    return nc


def _get_nc(reps: int = 1, timing: bool = False):
    key = ("nc", reps, timing)
    if key not in _NC_CACHE:
        _NC_CACHE[key] = _build_bass(reps, timing)
    return _NC_CACHE[key]


def kernel(z: np.ndarray, x: np.ndarray, _trace: bool = False):
    from concourse.bass_utils import run_bass_kernel_spmd

    z = np.ascontiguousarray(z, dtype=np.float32)
    x = np.ascontiguousarray(x, dtype=np.float32)
    assert z.shape == (B, C, HZ, WZ) and x.shape == (B, C, HX, WX)

    nc = _get_nc()
    in_maps = []
    for c in range(N_CORES):
        b0 = c * B_PER_CORE
        in_maps.append({
            "xs": x[b0:b0 + B_PER_CORE].reshape(PAIRS, XF),
            "zs": z[b0:b0 + B_PER_CORE].reshape(PAIRS, ZF),
        })
    res = run_bass_kernel_spmd(nc, in_maps, list(range(N_CORES)), trace=_trace)
    out = np.empty((B, C, HO, WO), dtype=np.float32)
    for c in range(N_CORES):
        b0 = c * B_PER_CORE
        out[b0:b0 + B_PER_CORE] = res.results[c]["out"].reshape(
            B_PER_CORE, C, HO, WO)
    if _trace:
        return out, res
    return out
